# revision 31
# baseline (speedup 1.0000x reference)
"""Trainium2 Bass kernel for the DynamicInnerLoop problem.

Algorithm (exact algebraic collapse of the reference scan):
  The reference runs 10 steps; each step evaluates two 3-layer MLPs whose
  first layer is a 128x(N+64) GEMV against concat(params, enc).  Since
  params only changes by scalar multiples of `gradients`
  (params_{t+1} = params_t - m_t*ss_t*gradients), the first-layer
  pre-activations evolve as  a_{t+1} = a_t - m_t*ss_t*gv  where
  a_0 = W1[:, :N]@p0 + W1[:, N:]@enc + b1  and  gv = W1[:, :N]@gradients.
  So the big weights are read exactly once, and the 10-step loop runs on
  128-dim vectors.  Further, because an inactive step freezes params, the
  active-gating can be moved OUT of the loop: run the recurrence
  unconditionally (a += -0.1*ss_t*gv), record the per-step logits, then
  compute stop flags / the active mask / S with a handful of vectorized
  ops.  Finally params_out = p0 - 0.1*S*gradients, count = sum(active).

Sharding: columns of both W1 matrices are split across 8 cores (25000
each, zero-padded to 25088 = 196*128).  Each core computes partial
(a0, gv) for both MLPs with x-stationary bf16 matmuls, the 2KB partials
are exchanged with a collective, and the tiny recurrence runs replicated
on every core; each core then updates its own shard of params in fp32.

The big weight stream is bf16 (halves HBM traffic and doubles PE
streaming rate); the fp32 anchors (p0, gradients, final update, a0
reduction, S) stay fp32, so the output error is ~1e-5 relative.
"""

import numpy as np
import ml_dtypes

import concourse.bass as bass
import concourse.bacc as bacc
import concourse.tile as tile
from concourse import mybir
from concourse.bass_utils import run_bass_kernel_spmd

# ---------------------------------------------------------------- constants
NCORES = 8
N = 200000
NS = N // NCORES            # 25000 params per core
P = 128
C = NS // P + (1 if NS % P else 0)  # 196 chunks of 128
NSP = C * P                 # 25088 padded shard length
MAX_STEPS = 10
CTX = 100

WCOLS = C * 2 * P           # 50176 columns of the combined bf16 wt tensor
NDMA = 2                    # two big weight DMAs -> 50 KB lines
TILE_COLS = WCOLS // NDMA   # 25088
CHUNKS_PER_TILE = C // NDMA  # 98

COLLECTIVE = "ag"           # "ag" (AllGather+reduce-mm) or "ar" (AllReduce)

F32 = mybir.dt.float32
BF16 = mybir.dt.bfloat16
I32 = mybir.dt.int32
BF = ml_dtypes.bfloat16

# packed_bf16 layout (columns)
XB = 0                      # x interleaved (c,j): 392
W2B = XB + 2 * C            # w2cat: 128
W3B = W2B + P               # w3cat ([sp;st] stacked): 1
TLB = W3B + 1               # tail (65 partitions used): 256
W3S = TLB + 2 * P           # st_w3 at partitions 0:64: 1
PKB_COLS = W3S + 1          # 778

# packed_f32 layout (columns)
XPP = 0                     # x_pg params: 196
XPG = XPP + C               # x_pg grads: 196
B2C = XPG + C               # b2cat: 1
CW1 = B2C + 1               # ce_w1.T (100 partitions): 64
CW2 = CW1 + 64              # ce_w2.T (64 partitions): 64
CTXC = CW2 + 64             # context (100 partitions): 1
CB1C = CTXC + 1             # ce_b1 (64 partitions): 1
CB2C = CB1C + 1             # ce_b2 (64 partitions): 1
B3C = CB2C + 1              # b3 row (1 partition): 2  [sp, st]
RVC = B3C + 2               # r' row (1 partition): 10
SELA = RVC + MAX_STEPS      # (a,g)x(sp,st) selectors (32 partitions): 4
B2S = SELA + 4              # st_b2 at partitions 0:64: 1
PKF_COLS = B2S + 1          # 671


def _build_program(collective=COLLECTIVE):
    nc = bacc.Bacc(
        "TRN2",
        target_bir_lowering=False,
        debug=False,
        num_devices=NCORES,
    )

    wt = nc.dram_tensor("wt", [P, WCOLS], BF16, kind="ExternalInput")
    pkb = nc.dram_tensor("pkb", [P, PKB_COLS], BF16, kind="ExternalInput")
    pkf = nc.dram_tensor("pkf", [P, PKF_COLS], F32, kind="ExternalInput")
    out_params = nc.dram_tensor("out_params", [P, C], F32, kind="ExternalOutput")
    out_count = nc.dram_tensor("out_count", [1, 1], I32, kind="ExternalOutput")

    with tile.TileContext(nc) as tc:
        with (
            tc.tile_pool(name="wpool", bufs=2) as wpool,
            tc.tile_pool(name="sbuf", bufs=1) as sb,
            tc.tile_pool(name="psum_gemv", bufs=1, space="PSUM") as pg,
            tc.tile_pool(name="psum_small", bufs=3, space="PSUM") as ps,
            tc.tile_pool(name="psum_hold", bufs=1, space="PSUM") as ph,
            tc.tile_pool(name="dram", bufs=1, space="DRAM") as dram,
        ):
            pkb_t = sb.tile([P, PKB_COLS], BF16)
            pkf_t = sb.tile([P, PKF_COLS], F32)
            nc.sync.dma_start(pkb_t[:], pkb[:])
            nc.sync.dma_start(pkf_t[:], pkf[:])

            # ----------------------------------------------- constants
            enc2 = sb.tile([P // 2 + 1, 2], BF16)     # [enc;1 | 0]
            nc.vector.memset(enc2[:], 0.0)
            nc.vector.memset(enc2[64:65, 0:1], 1.0)
            zcol = sb.tile([P, 1], F32)
            nc.vector.memset(zcol[:], 0.0)
            ones_row = sb.tile([1, P], F32)
            nc.vector.memset(ones_row[:], 1.0)

            # prewarm the ACT sigmoid table so the loop's first sigmoid
            # doesn't pay the ~1.3us table load
            warm = sb.tile([1, 1], F32)
            nc.vector.memset(warm[:], 0.0)
            warm2 = sb.tile([1, 1], F32)
            nc.scalar.activation(warm2[:], warm[:],
                                 mybir.ActivationFunctionType.Sigmoid)

            # ----------------------------------------------- context encoder
            p_ce1 = ps.tile([64, 1], F32, tag="small")
            nc.tensor.matmul(p_ce1[:], pkf_t[0:CTX, CW1:CW1 + 64],
                             pkf_t[0:CTX, CTXC:CTXC + 1], start=True, stop=True)
            s1 = sb.tile([64, 1], F32)
            nc.scalar.activation(
                s1[:], p_ce1[:], mybir.ActivationFunctionType.Relu,
                bias=pkf_t[0:64, CB1C:CB1C + 1],
            )
            p_ce2 = ps.tile([64, 1], F32, tag="small")
            nc.tensor.matmul(p_ce2[:], pkf_t[0:64, CW2:CW2 + 64], s1[:],
                             start=True, stop=True)
            nc.scalar.activation(
                enc2[0:64, 0:1], p_ce2[:], mybir.ActivationFunctionType.Identity,
                bias=pkf_t[0:64, CB2C:CB2C + 1],
            )

            # ----------------------------------------------- big GEMV stream
            psum_cat = pg.tile([2, 2 * P], F32)
            for d in range(NDMA):
                wt_t = wpool.tile([P, TILE_COLS], BF16)
                nc.sync.dma_start(
                    wt_t[:], wt[:, d * TILE_COLS:(d + 1) * TILE_COLS]
                )
                for j in range(CHUNKS_PER_TILE):
                    c = d * CHUNKS_PER_TILE + j
                    nc.tensor.matmul(
                        psum_cat[:],
                        pkb_t[:, 2 * c:2 * c + 2],
                        wt_t[:, j * 2 * P:(j + 1) * 2 * P],
                        start=(c == 0),
                        stop=False,
                    )
            # e/bias contribution closes the group
            nc.tensor.matmul(psum_cat[:], enc2[:], pkb_t[0:65, TLB:TLB + 2 * P],
                             start=False, stop=True)

            # ----------------------------------------------- exchange partials
            cat_sb = sb.tile([2, 2 * P], F32)
            nc.scalar.copy(cat_sb[:], psum_cat[:])
            cc_in = dram.tile([4, P], F32)
            # psum_cat rows (j, s*128+n) map to cc_in rows 2j+s.
            nc.sync.dma_start(
                cc_in[:].rearrange("(j s) n -> j (s n)", j=2), cat_sb[:])

            if collective == "ag":
                cc_out = dram.tile([4 * NCORES, P], F32)
                nc.gpsimd.collective_compute(
                    "AllGather",
                    mybir.AluOpType.bypass,
                    replica_groups=[list(range(NCORES))],
                    ins=[cc_in.opt()],
                    outs=[cc_out.opt()],
                )
                krows = 4 * NCORES
            else:
                cc_out = dram.tile([4, P], F32)
                nc.gpsimd.collective_compute(
                    "AllReduce",
                    mybir.AluOpType.add,
                    replica_groups=[list(range(NCORES))],
                    ins=[cc_in.opt()],
                    outs=[cc_out.opt()],
                )
                krows = 4
            g_sb = sb.tile([krows, P], F32)
            nc.sync.dma_start(g_sb[:], cc_out[:])
            # reduce + transpose to columns: cat4[m, s, kind] (kind: a=0, g=1)
            psum_ag = ph.tile([P, 2, 2], F32)
            nc.tensor.matmul(
                psum_ag[:].rearrange("m s k -> m (s k)"),
                g_sb[:], pkf_t[0:krows, SELA:SELA + 4], start=True, stop=True)
            cat4 = sb.tile([P, 2, 2], F32)
            nc.scalar.copy(cat4[:], psum_ag[:])
            a_v = cat4[:, :, 0]                       # [128, 2] a state (fp32)
            g_v = cat4[:, :, 1]                       # [128, 2] gv columns
            neg01_bf = sb.tile([1, P], BF16)
            nc.vector.memset(neg01_bf[:], -0.1)

            # ----------------------------------------------- phase A: 10 steps
            # unconditional recurrence; per-step logits land in psum3_all
            psum3_all = ph.tile([1, MAX_STEPS, 2], F32)
            b3sp = pkf_t[0:1, B3C:B3C + 1]
            b3st = pkf_t[0:1, B3C + 1:B3C + 2]
            for t in range(MAX_STEPS):
                # critical sp chain: u -> h2 -> logit -> sigmoid -> a update
                u_sp = sb.tile([P, 1], BF16, tag="usp")
                nc.vector.tensor_scalar_max(u_sp[:], a_v[:, 0:1], 0.0)
                # early read of the st column (rest of the st path lags)
                u_st = sb.tile([P, 1], BF16, tag="ust")
                nc.vector.tensor_scalar_max(u_st[:], a_v[:, 1:2], 0.0)

                p2sp = ps.tile([64, 1], F32, tag="small")
                nc.tensor.matmul(p2sp[:], pkb_t[:, W2B:W2B + 64], u_sp[:],
                                 start=True, stop=True)
                h2sp = sb.tile([64, 1], BF16, tag="h2sp")
                nc.scalar.activation(
                    h2sp[:], p2sp[:], mybir.ActivationFunctionType.Relu,
                    bias=pkf_t[0:64, B2C:B2C + 1],
                )
                nc.tensor.matmul(psum3_all[:, t, 0:1], pkb_t[0:64, W3B:W3B + 1],
                                 h2sp[:], start=True, stop=True)
                sig_bf = sb.tile([1, 1], BF16, tag="sig")
                nc.scalar.activation(
                    sig_bf[:], psum3_all[:, t, 0:1],
                    mybir.ActivationFunctionType.Sigmoid, bias=b3sp,
                )
                # a += (-0.1*ss) * gv  (scalar broadcast via 1x128 matmul)
                pb = ps.tile([P, 1], F32, tag="small")
                nc.tensor.matmul(pb[:], neg01_bf[:], sig_bf[:],
                                 start=True, stop=True)
                nc.vector.scalar_tensor_tensor(
                    a_v, g_v, pb[:], a_v,
                    op0=mybir.AluOpType.mult, op1=mybir.AluOpType.add,
                )

                # lagging st path: only feeds phase-B stop probabilities
                p2st = ps.tile([64, 1], F32, tag="small")
                nc.tensor.matmul(p2st[:], pkb_t[:, W2B + 64:W2B + P], u_st[:],
                                 start=True, stop=True)
                h2st = sb.tile([64, 1], BF16, tag="h2st")
                nc.vector.scalar_tensor_tensor(
                    h2st[:], p2st[:], pkf_t[0:64, B2S:B2S + 1], zcol[0:64, 0:1],
                    op0=mybir.AluOpType.add, op1=mybir.AluOpType.max,
                )
                nc.tensor.matmul(psum3_all[:, t, 1:2], pkb_t[0:64, W3S:W3S + 1],
                                 h2st[:], start=True, stop=True)

            # ----------------------------------------------- phase B: gating
            ss_row = sb.tile([1, MAX_STEPS], F32)
            nc.scalar.activation(ss_row[:], psum3_all[:, :, 0],
                                 mybir.ActivationFunctionType.Sigmoid, bias=b3sp)
            pstop_row = sb.tile([1, MAX_STEPS], F32)
            nc.scalar.activation(pstop_row[:], psum3_all[:, :, 1],
                                 mybir.ActivationFunctionType.Sigmoid, bias=b3st)
            notstop = sb.tile([1, MAX_STEPS], F32)
            nc.vector.tensor_tensor(
                notstop[:], pkf_t[0:1, RVC:RVC + MAX_STEPS], pstop_row[:],
                op=mybir.AluOpType.is_ge,
            )
            # act_t = prod_{tau<t} notstop_tau (tensor_tensor_scan and
            # tensor_tensor_reduce both crash the exec unit on this HW
            # config, so use plain serial DVE ops — it's only 10 steps)
            act_row = sb.tile([1, MAX_STEPS], F32)
            nc.vector.memset(act_row[:, 0:1], 1.0)
            for t in range(1, MAX_STEPS):
                nc.vector.tensor_tensor(
                    act_row[:, t:t + 1], act_row[:, t - 1:t],
                    notstop[:, t - 1:t], op=mybir.AluOpType.mult,
                )
            # S_neg = -0.1 * sum(act*ss); count = sum(act)
            sprod = sb.tile([1, MAX_STEPS], F32)
            nc.vector.tensor_tensor(sprod[:], act_row[:], ss_row[:],
                                    op=mybir.AluOpType.mult)
            ssum = sb.tile([1, 1], F32)
            nc.vector.tensor_reduce(ssum[:], sprod[:],
                                    axis=mybir.AxisListType.X,
                                    op=mybir.AluOpType.add)
            s_neg = sb.tile([1, 1], F32)
            nc.scalar.mul(s_neg[:], ssum[:], -0.1)
            cnt_f = sb.tile([1, 1], F32)
            nc.vector.tensor_reduce(cnt_f[:], act_row[:],
                                    axis=mybir.AxisListType.X,
                                    op=mybir.AluOpType.add)
            cnt_i = sb.tile([1, 1], I32)
            nc.vector.tensor_copy(cnt_i[:], cnt_f[:])
            nc.sync.dma_start(out_count[:], cnt_i[:])

            # ----------------------------------------------- final update
            pS = ps.tile([P, 1], F32, tag="small")
            nc.tensor.matmul(pS[:], ones_row[:], s_neg[:], start=True, stop=True)
            out_t = sb.tile([P, C], F32)
            nc.vector.scalar_tensor_tensor(
                out_t[:], pkf_t[:, XPG:XPG + C], pS[:], pkf_t[:, XPP:XPP + C],
                op0=mybir.AluOpType.mult, op1=mybir.AluOpType.add,
            )
            nc.sync.dma_start(out_params[:], out_t[:])

    nc.compile()
    return nc


_COMPILED = {}


def _get_program(collective=COLLECTIVE):
    if collective not in _COMPILED:
        _COMPILED[collective] = _build_program(collective)
    return _COMPILED[collective]


def _prep_inputs(inputs):
    """Build the 8 per-core input maps from full-size numpy inputs."""
    f = lambda k: np.asarray(inputs[k], dtype=np.float32)
    p0, g = f("initial_params"), f("gradients")
    sp_w1, st_w1 = f("sp_w1"), f("st_w1")

    # ---- shared packed fp32 tensor
    pkf = np.zeros((P, PKF_COLS), np.float32)
    pkf[0:64, B2C] = f("sp_b2")
    pkf[64:128, B2C] = f("st_b2")
    pkf[0:64, B2S] = f("st_b2")
    pkf[0:CTX, CW1:CW1 + 64] = f("ce_w1").T
    pkf[0:64, CW2:CW2 + 64] = f("ce_w2").T
    pkf[0:CTX, CTXC] = f("context")
    pkf[0:64, CB1C] = f("ce_b1")
    pkf[0:64, CB2C] = f("ce_b2")
    pkf[0, B3C] = f("sp_b3")[0]
    pkf[0, B3C + 1] = f("st_b3")[0]
    rv = f("rand_vals").copy()
    rv[0] = 2.0  # MIN_STEPS=1: step 0 can never stop
    pkf[0, RVC:RVC + MAX_STEPS] = rv
    # cc rows (per rank) are 2*kind+s; cat4 column n = 2*s+kind
    for k in range(4 * NCORES):
        kind, s = (k % 4) // 2, (k % 4) % 2
        pkf[k, SELA + 2 * s + kind] = 1.0
    pkf_shared = pkf

    # ---- shared part of packed bf16 tensor (w2cat/w3cat/tail)
    pkb_shared = np.zeros((P, PKB_COLS), BF)
    pkb_shared[:, W2B:W2B + P] = np.concatenate(
        [f("sp_w2").T, f("st_w2").T], axis=1).astype(BF)
    pkb_shared[0:64, W3B] = f("sp_w3")[0].astype(BF)
    pkb_shared[64:128, W3B] = f("st_w3")[0].astype(BF)
    pkb_shared[0:64, W3S] = f("st_w3")[0].astype(BF)
    tail_sp = np.concatenate(
        [sp_w1[:, N:].T, f("sp_b1")[None, :]], axis=0) / NCORES  # [65,128]
    tail_st = np.concatenate(
        [st_w1[:, N:].T, f("st_b1")[None, :]], axis=0) / NCORES
    pkb_shared[0:65, TLB:TLB + 2 * P] = np.concatenate(
        [tail_sp, tail_st], axis=1).astype(BF)

    in_maps = []
    for r in range(NCORES):
        sl = slice(r * NS, (r + 1) * NS)
        # wt[k, (c, s, m)] = W_s[m, r*NS + c*128 + k]
        ws = np.zeros((2, P, NSP), np.float32)
        ws[0, :, :NS] = sp_w1[:, sl]
        ws[1, :, :NS] = st_w1[:, sl]
        wt = np.ascontiguousarray(
            ws.reshape(2, P, C, P).transpose(3, 2, 0, 1)).reshape(P, WCOLS)

        p_pad = np.zeros(NSP, np.float32)
        p_pad[:NS] = p0[sl]
        g_pad = np.zeros(NSP, np.float32)
        g_pad[:NS] = g[sl]

        pkb_r = pkb_shared.copy()
        pkb_r[:, XB:XB + 2 * C] = np.stack(
            [p_pad.reshape(C, P).T, g_pad.reshape(C, P).T], axis=2
        ).reshape(P, 2 * C).astype(BF)

        pkf_r = pkf_shared.copy()
        pkf_r[:, XPP:XPP + C] = p_pad.reshape(C, P).T
        pkf_r[:, XPG:XPG + C] = g_pad.reshape(C, P).T

        in_maps.append({
            "wt": wt.astype(BF),
            "pkb": pkb_r,
            "pkf": pkf_r,
        })
    return in_maps


LAST_RESULTS = None  # BassKernelResults of the most recent run (for test.py)


def kernel(trace=False, collective=COLLECTIVE, trace_cores=None, **inputs):
    global LAST_RESULTS
    nc = _get_program(collective)
    in_maps = _prep_inputs(inputs)
    kw = {}
    if trace_cores is not None:
        kw["trace_cores"] = trace_cores
    res = run_bass_kernel_spmd(
        nc, in_maps, list(range(NCORES)), trace=trace, **kw,
    )
    LAST_RESULTS = res
    outs = res.results
    params = np.empty(N, np.float32)
    for r in range(NCORES):
        shard = outs[r]["out_params"]  # [128, 196], (k, c) = p[c*128+k]
        params[r * NS:(r + 1) * NS] = shard.T.reshape(-1)[:NS]
    count = np.int32(outs[0]["out_count"].reshape(-1)[0])
    return params, count


# revision 32
# speedup vs baseline: 1.3166x; 1.3166x over previous
"""Trainium2 Bass kernel for the DynamicInnerLoop problem.

Algorithm (exact algebraic collapse of the reference scan):
  The reference runs 10 steps; each step evaluates two 3-layer MLPs whose
  first layer is a 128x(N+64) GEMV against concat(params, enc).  Since
  params only changes by scalar multiples of `gradients`
  (params_{t+1} = params_t - m_t*ss_t*gradients), the first-layer
  pre-activations evolve as  a_{t+1} = a_t - m_t*ss_t*gv  where
  a_0 = W1[:, :N]@p0 + W1[:, N:]@enc + b1  and  gv = W1[:, :N]@gradients.
  So the big weights are read exactly once, and the 10-step loop runs on
  128-dim vectors.  Further, because an inactive step freezes params, the
  active-gating can be moved OUT of the loop: run the recurrence
  unconditionally (a += -0.1*ss_t*gv), record the per-step logits, then
  compute stop flags / the active mask / S with a handful of vectorized
  ops.  Finally params_out = p0 - 0.1*S*gradients, count = sum(active).

Sharding: columns of both W1 matrices are split across 8 cores (25000
each, zero-padded to 25088 = 196*128).  Each core computes partial
(a0, gv) for both MLPs with x-stationary bf16 matmuls, the 2KB partials
are exchanged with a collective, and the tiny recurrence runs replicated
on every core; each core then updates its own shard of params in fp32.

The big weight stream is bf16 (halves HBM traffic and doubles PE
streaming rate); the fp32 anchors (p0, gradients, final update, a0
reduction, S) stay fp32, so the output error is ~1e-5 relative.
"""

import numpy as np
import ml_dtypes

import concourse.bass as bass
import concourse.bacc as bacc
import concourse.tile as tile
from concourse import mybir
from concourse.bass_utils import run_bass_kernel_spmd

# ---------------------------------------------------------------- constants
NCORES = 8
N = 200000
NS = N // NCORES            # 25000 params per core
P = 128
C = NS // P + (1 if NS % P else 0)  # 196 chunks of 128
NSP = C * P                 # 25088 padded shard length
MAX_STEPS = 10
CTX = 100

WCOLS = C * 2 * P           # 50176 columns of the combined bf16 wt tensor
NDMA = 2                    # two big weight DMAs -> 50 KB lines
TILE_COLS = WCOLS // NDMA   # 25088
CHUNKS_PER_TILE = C // NDMA  # 98

COLLECTIVE = "ag"           # "ag" (AllGather+reduce-mm) or "ar" (AllReduce)

F32 = mybir.dt.float32
BF16 = mybir.dt.bfloat16
I32 = mybir.dt.int32
BF = ml_dtypes.bfloat16

# packed_bf16 layout (columns)
XB = 0                      # x interleaved (c,j): 392
W2B = XB + 2 * C            # w2cat: 128
W3B = W2B + P               # w3cat ([sp;st] stacked): 1
TLB = W3B + 1               # tail (65 partitions used): 256
W3S = TLB + 2 * P           # st_w3 at partitions 0:64: 1
PKB_COLS = W3S + 1          # 778

# packed_f32 layout (columns)
XPP = 0                     # x_pg params: 196
XPG = XPP + C               # x_pg grads: 196
B2C = XPG + C               # b2cat: 1
CW1 = B2C + 1               # ce_w1.T (100 partitions): 64
CW2 = CW1 + 64              # ce_w2.T (64 partitions): 64
CTXC = CW2 + 64             # context (100 partitions): 1
CB1C = CTXC + 1             # ce_b1 (64 partitions): 1
CB2C = CB1C + 1             # ce_b2 (64 partitions): 1
B3C = CB2C + 1              # b3 row (1 partition): 2  [sp, st]
RVC = B3C + 2               # r' row (1 partition): 10
SELA = RVC + MAX_STEPS      # (a,g)x(sp,st) selectors (32 partitions): 4
PKF_COLS = SELA + 4         # 670


def _build_program(collective=COLLECTIVE):
    nc = bacc.Bacc(
        "TRN2",
        target_bir_lowering=False,
        debug=False,
        num_devices=NCORES,
    )

    wt = nc.dram_tensor("wt", [P, WCOLS], BF16, kind="ExternalInput")
    pkb = nc.dram_tensor("pkb", [P, PKB_COLS], BF16, kind="ExternalInput")
    pkf = nc.dram_tensor("pkf", [P, PKF_COLS], F32, kind="ExternalInput")
    out_params = nc.dram_tensor("out_params", [P, C], F32, kind="ExternalOutput")
    out_count = nc.dram_tensor("out_count", [1, 1], I32, kind="ExternalOutput")

    with tile.TileContext(nc) as tc:
        with (
            tc.tile_pool(name="wpool", bufs=2) as wpool,
            tc.tile_pool(name="sbuf", bufs=1) as sb,
            tc.tile_pool(name="psum_gemv", bufs=1, space="PSUM") as pg,
            tc.tile_pool(name="psum_small", bufs=3, space="PSUM") as ps,
            tc.tile_pool(name="psum_hold", bufs=1, space="PSUM") as ph,
            tc.tile_pool(name="dram", bufs=1, space="DRAM") as dram,
        ):
            pkb_t = sb.tile([P, PKB_COLS], BF16)
            pkf_t = sb.tile([P, PKF_COLS], F32)
            nc.sync.dma_start(pkb_t[:], pkb[:])
            nc.sync.dma_start(pkf_t[:], pkf[:])

            # ----------------------------------------------- constants
            enc2 = sb.tile([P // 2 + 1, 2], BF16)     # [enc;1 | 0]
            nc.vector.memset(enc2[:], 0.0)
            nc.vector.memset(enc2[64:65, 0:1], 1.0)
            h2z = sb.tile([P, 2], BF16)               # zero-masked layer2 act
            nc.vector.memset(h2z[:], 0.0)
            zcol = sb.tile([P, 1], F32)
            nc.vector.memset(zcol[:], 0.0)
            ones_row = sb.tile([1, P], F32)
            nc.vector.memset(ones_row[:], 1.0)

            # prewarm the ACT sigmoid table so the loop's first sigmoid
            # doesn't pay the ~1.3us table load
            warm = sb.tile([1, 1], F32)
            nc.vector.memset(warm[:], 0.0)
            warm2 = sb.tile([1, 1], F32)
            nc.scalar.activation(warm2[:], warm[:],
                                 mybir.ActivationFunctionType.Sigmoid)

            # ----------------------------------------------- context encoder
            p_ce1 = ps.tile([64, 1], F32, tag="small")
            nc.tensor.matmul(p_ce1[:], pkf_t[0:CTX, CW1:CW1 + 64],
                             pkf_t[0:CTX, CTXC:CTXC + 1], start=True, stop=True)
            s1 = sb.tile([64, 1], F32)
            nc.scalar.activation(
                s1[:], p_ce1[:], mybir.ActivationFunctionType.Relu,
                bias=pkf_t[0:64, CB1C:CB1C + 1],
            )
            p_ce2 = ps.tile([64, 1], F32, tag="small")
            nc.tensor.matmul(p_ce2[:], pkf_t[0:64, CW2:CW2 + 64], s1[:],
                             start=True, stop=True)
            nc.scalar.activation(
                enc2[0:64, 0:1], p_ce2[:], mybir.ActivationFunctionType.Identity,
                bias=pkf_t[0:64, CB2C:CB2C + 1],
            )

            # ----------------------------------------------- big GEMV stream
            psum_cat = pg.tile([2, 2 * P], F32)
            for d in range(NDMA):
                wt_t = wpool.tile([P, TILE_COLS], BF16)
                nc.sync.dma_start(
                    wt_t[:], wt[:, d * TILE_COLS:(d + 1) * TILE_COLS]
                )
                for j in range(CHUNKS_PER_TILE):
                    c = d * CHUNKS_PER_TILE + j
                    nc.tensor.matmul(
                        psum_cat[:],
                        pkb_t[:, 2 * c:2 * c + 2],
                        wt_t[:, j * 2 * P:(j + 1) * 2 * P],
                        start=(c == 0),
                        stop=False,
                    )
            # e/bias contribution closes the group
            nc.tensor.matmul(psum_cat[:], enc2[:], pkb_t[0:65, TLB:TLB + 2 * P],
                             start=False, stop=True)

            # ----------------------------------------------- exchange partials
            cat_sb = sb.tile([2, 2 * P], F32)
            nc.scalar.copy(cat_sb[:], psum_cat[:])
            cc_in = dram.tile([4, P], F32)
            # psum_cat rows (j, s*128+n) map to cc_in rows 2j+s.
            nc.sync.dma_start(
                cc_in[:].rearrange("(j s) n -> j (s n)", j=2), cat_sb[:])

            if collective == "ag":
                cc_out = dram.tile([4 * NCORES, P], F32)
                nc.gpsimd.collective_compute(
                    "AllGather",
                    mybir.AluOpType.bypass,
                    replica_groups=[list(range(NCORES))],
                    ins=[cc_in.opt()],
                    outs=[cc_out.opt()],
                )
                krows = 4 * NCORES
            else:
                cc_out = dram.tile([4, P], F32)
                nc.gpsimd.collective_compute(
                    "AllReduce",
                    mybir.AluOpType.add,
                    replica_groups=[list(range(NCORES))],
                    ins=[cc_in.opt()],
                    outs=[cc_out.opt()],
                )
                krows = 4
            g_sb = sb.tile([krows, P], F32)
            nc.sync.dma_start(g_sb[:], cc_out[:])
            # reduce + transpose to columns: cat4[m, s, kind] (kind: a=0, g=1)
            psum_ag = ph.tile([P, 2, 2], F32)
            nc.tensor.matmul(
                psum_ag[:].rearrange("m s k -> m (s k)"),
                g_sb[:], pkf_t[0:krows, SELA:SELA + 4], start=True, stop=True)
            cat4 = sb.tile([P, 2, 2], F32)
            nc.scalar.copy(cat4[:], psum_ag[:])
            a_v = cat4[:, :, 0]                       # [128, 2] a state (fp32)
            g_v = cat4[:, :, 1]                       # [128, 2] gv columns
            neg01_bf = sb.tile([1, P], BF16)
            nc.vector.memset(neg01_bf[:], -0.1)

            # ----------------------------------------------- phase A: 10 steps
            # unconditional recurrence; per-step logits land in psum3_all
            psum3_all = ph.tile([1, MAX_STEPS, 2], F32)
            b3sp = pkf_t[0:1, B3C:B3C + 1]
            b3st = pkf_t[0:1, B3C + 1:B3C + 2]
            for t in range(MAX_STEPS):
                u = sb.tile([P, 2], BF16, tag="u")
                nc.vector.tensor_scalar_max(u[:], a_v, 0.0)

                p2 = ps.tile([P, 2], F32, tag="small")
                nc.tensor.matmul(p2[:], pkb_t[:, W2B:W2B + P], u[:],
                                 start=True, stop=True)
                # h2z halves: relu(p2 + b2); off-diagonal halves stay zero
                nc.scalar.activation(
                    h2z[0:64, 0:1], p2[0:64, 0:1],
                    mybir.ActivationFunctionType.Relu,
                    bias=pkf_t[0:64, B2C:B2C + 1],
                )
                nc.vector.scalar_tensor_tensor(
                    h2z[64:128, 1:2], p2[64:128, 1:2],
                    pkf_t[64:128, B2C:B2C + 1], zcol[64:128, 0:1],
                    op0=mybir.AluOpType.add, op1=mybir.AluOpType.max,
                )

                nc.tensor.matmul(psum3_all[:, t, :], pkb_t[:, W3B:W3B + 1],
                                 h2z[:], start=True, stop=True)

                sig_bf = sb.tile([1, 1], BF16, tag="sig")
                nc.scalar.activation(
                    sig_bf[:], psum3_all[:, t, 0:1],
                    mybir.ActivationFunctionType.Sigmoid, bias=b3sp,
                )
                # a += (-0.1*ss) * gv  (scalar broadcast via 1x128 matmul)
                pb = ps.tile([P, 1], F32, tag="small")
                nc.tensor.matmul(pb[:], neg01_bf[:], sig_bf[:],
                                 start=True, stop=True)
                nc.vector.scalar_tensor_tensor(
                    a_v, g_v, pb[:], a_v,
                    op0=mybir.AluOpType.mult, op1=mybir.AluOpType.add,
                )

            # ----------------------------------------------- phase B: gating
            ss_row = sb.tile([1, MAX_STEPS], F32)
            nc.scalar.activation(ss_row[:], psum3_all[:, :, 0],
                                 mybir.ActivationFunctionType.Sigmoid, bias=b3sp)
            pstop_row = sb.tile([1, MAX_STEPS], F32)
            nc.scalar.activation(pstop_row[:], psum3_all[:, :, 1],
                                 mybir.ActivationFunctionType.Sigmoid, bias=b3st)
            notstop = sb.tile([1, MAX_STEPS], F32)
            nc.vector.tensor_tensor(
                notstop[:], pkf_t[0:1, RVC:RVC + MAX_STEPS], pstop_row[:],
                op=mybir.AluOpType.is_ge,
            )
            # act_t = prod_{tau<t} notstop_tau (tensor_tensor_scan and
            # tensor_tensor_reduce both crash the exec unit on this HW
            # config, so use plain serial DVE ops — it's only 10 steps)
            act_row = sb.tile([1, MAX_STEPS], F32)
            nc.vector.memset(act_row[:, 0:1], 1.0)
            for t in range(1, MAX_STEPS):
                nc.vector.tensor_tensor(
                    act_row[:, t:t + 1], act_row[:, t - 1:t],
                    notstop[:, t - 1:t], op=mybir.AluOpType.mult,
                )
            # S_neg = -0.1 * sum(act*ss); count = sum(act)
            sprod = sb.tile([1, MAX_STEPS], F32)
            nc.vector.tensor_tensor(sprod[:], act_row[:], ss_row[:],
                                    op=mybir.AluOpType.mult)
            ssum = sb.tile([1, 1], F32)
            nc.vector.tensor_reduce(ssum[:], sprod[:],
                                    axis=mybir.AxisListType.X,
                                    op=mybir.AluOpType.add)
            s_neg = sb.tile([1, 1], F32)
            nc.scalar.mul(s_neg[:], ssum[:], -0.1)
            cnt_f = sb.tile([1, 1], F32)
            nc.vector.tensor_reduce(cnt_f[:], act_row[:],
                                    axis=mybir.AxisListType.X,
                                    op=mybir.AluOpType.add)
            cnt_i = sb.tile([1, 1], I32)
            nc.vector.tensor_copy(cnt_i[:], cnt_f[:])
            nc.sync.dma_start(out_count[:], cnt_i[:])

            # ----------------------------------------------- final update
            pS = ps.tile([P, 1], F32, tag="small")
            nc.tensor.matmul(pS[:], ones_row[:], s_neg[:], start=True, stop=True)
            out_t = sb.tile([P, C], F32)
            nc.vector.scalar_tensor_tensor(
                out_t[:], pkf_t[:, XPG:XPG + C], pS[:], pkf_t[:, XPP:XPP + C],
                op0=mybir.AluOpType.mult, op1=mybir.AluOpType.add,
            )
            nc.sync.dma_start(out_params[:], out_t[:])

    nc.compile()
    return nc


_COMPILED = {}


def _get_program(collective=COLLECTIVE):
    if collective not in _COMPILED:
        _COMPILED[collective] = _build_program(collective)
    return _COMPILED[collective]


def _prep_inputs(inputs):
    """Build the 8 per-core input maps from full-size numpy inputs."""
    f = lambda k: np.asarray(inputs[k], dtype=np.float32)
    p0, g = f("initial_params"), f("gradients")
    sp_w1, st_w1 = f("sp_w1"), f("st_w1")

    # ---- shared packed fp32 tensor
    pkf = np.zeros((P, PKF_COLS), np.float32)
    pkf[0:64, B2C] = f("sp_b2")
    pkf[64:128, B2C] = f("st_b2")
    pkf[0:CTX, CW1:CW1 + 64] = f("ce_w1").T
    pkf[0:64, CW2:CW2 + 64] = f("ce_w2").T
    pkf[0:CTX, CTXC] = f("context")
    pkf[0:64, CB1C] = f("ce_b1")
    pkf[0:64, CB2C] = f("ce_b2")
    pkf[0, B3C] = f("sp_b3")[0]
    pkf[0, B3C + 1] = f("st_b3")[0]
    rv = f("rand_vals").copy()
    rv[0] = 2.0  # MIN_STEPS=1: step 0 can never stop
    pkf[0, RVC:RVC + MAX_STEPS] = rv
    # cc rows (per rank) are 2*kind+s; cat4 column n = 2*s+kind
    for k in range(4 * NCORES):
        kind, s = (k % 4) // 2, (k % 4) % 2
        pkf[k, SELA + 2 * s + kind] = 1.0
    pkf_shared = pkf

    # ---- shared part of packed bf16 tensor (w2cat/w3cat/tail)
    pkb_shared = np.zeros((P, PKB_COLS), BF)
    pkb_shared[:, W2B:W2B + P] = np.concatenate(
        [f("sp_w2").T, f("st_w2").T], axis=1).astype(BF)
    pkb_shared[0:64, W3B] = f("sp_w3")[0].astype(BF)
    pkb_shared[64:128, W3B] = f("st_w3")[0].astype(BF)
    tail_sp = np.concatenate(
        [sp_w1[:, N:].T, f("sp_b1")[None, :]], axis=0) / NCORES  # [65,128]
    tail_st = np.concatenate(
        [st_w1[:, N:].T, f("st_b1")[None, :]], axis=0) / NCORES
    pkb_shared[0:65, TLB:TLB + 2 * P] = np.concatenate(
        [tail_sp, tail_st], axis=1).astype(BF)

    in_maps = []
    for r in range(NCORES):
        sl = slice(r * NS, (r + 1) * NS)
        # wt[k, (c, s, m)] = W_s[m, r*NS + c*128 + k]
        ws = np.zeros((2, P, NSP), np.float32)
        ws[0, :, :NS] = sp_w1[:, sl]
        ws[1, :, :NS] = st_w1[:, sl]
        wt = np.ascontiguousarray(
            ws.reshape(2, P, C, P).transpose(3, 2, 0, 1)).reshape(P, WCOLS)

        p_pad = np.zeros(NSP, np.float32)
        p_pad[:NS] = p0[sl]
        g_pad = np.zeros(NSP, np.float32)
        g_pad[:NS] = g[sl]

        pkb_r = pkb_shared.copy()
        pkb_r[:, XB:XB + 2 * C] = np.stack(
            [p_pad.reshape(C, P).T, g_pad.reshape(C, P).T], axis=2
        ).reshape(P, 2 * C).astype(BF)

        pkf_r = pkf_shared.copy()
        pkf_r[:, XPP:XPP + C] = p_pad.reshape(C, P).T
        pkf_r[:, XPG:XPG + C] = g_pad.reshape(C, P).T

        in_maps.append({
            "wt": wt.astype(BF),
            "pkb": pkb_r,
            "pkf": pkf_r,
        })
    return in_maps


LAST_RESULTS = None  # BassKernelResults of the most recent run (for test.py)


def kernel(trace=False, collective=COLLECTIVE, trace_cores=None, **inputs):
    global LAST_RESULTS
    nc = _get_program(collective)
    in_maps = _prep_inputs(inputs)
    kw = {}
    if trace_cores is not None:
        kw["trace_cores"] = trace_cores
    res = run_bass_kernel_spmd(
        nc, in_maps, list(range(NCORES)), trace=trace, **kw,
    )
    LAST_RESULTS = res
    outs = res.results
    params = np.empty(N, np.float32)
    for r in range(NCORES):
        shard = outs[r]["out_params"]  # [128, 196], (k, c) = p[c*128+k]
        params[r * NS:(r + 1) * NS] = shard.T.reshape(-1)[:NS]
    count = np.int32(outs[0]["out_count"].reshape(-1)[0])
    return params, count
